# revision 4
# baseline (speedup 1.0000x reference)
"""Trainium2 Bass kernel for nn_ExperimentalMSELoss_17935783428185.

Reference math (pred, target: [64, 1, 512, 512] f32, uniform [0,1)):
    mask = target > 0.1
    i    = clip(target*mask, 1e-8)^0.001
    total_map = (pred*mask*i - target*mask*i)^2 + ((pred-target)*(1-mask))^2
              = (pred-target)^2 * (mask*target^0.002 + (1-mask))
    loss = total_map.sum()
         + 1e-3 * sum_b |max_b pred - max_b target| / numel      (~3e-19 rel)
         + 1e-3 * sum_b |sum_b pred - sum_b target| / numel      (~1e-11 rel)
         + 1e-3 * mean((hist10(pred) - hist10(target))^2)        (~2.5e-16 rel)

The three weighted terms are 8+ orders of magnitude below the f32 ULP of the
map-sum (~9e-8 relative); the reference's own f32 addition rounds the max and
hist terms away entirely. We compute the dominant map-sum exactly and the sum
term (it rides on a cheap fused accumulate); max/hist are omitted - verified
to change the f32 result by < 1e-11 relative.

Device computation, data-parallel over batch (8 samples per core, 8 cores).
Per sample tile [128, 2048], fp16 intermediates (weight e kept f32):
    u = max(target, 0.1)             GPSIMD tensor_scalar      (f16)
    m = target > 0.1                 GPSIMD tensor_scalar      (f16)
    v = ln(u)                        ACT                       (f16)
    p = m * v                        DVE tensor_tensor 2x      (f16)
    e = exp(0.001 * p)               ACT                       (f32)  e^2 = w
    d = pred - target                DVE tensor_tensor 2x      (f16)
    sd[:, s] = sum(d)                DVE tensor_scalar 4x + accum_out
    g = d * e                        DVE tensor_tensor (mixed) (f16)
    s2a[:, s] = sum(g[:, :Z]^2)      ACT Square + accum_out
    s2b[:, s] = sum(g[:, Z:]^2)      DVE scalar_tensor_tensor + accum_out
out[128, 24] = [s2a | s2b | sd];  host reduces in f64, casts to f32.

fp16 error budget (vs f32 reference): mask blur at the 0.1 threshold ~2e-6,
input-cast noise ~2e-7, fp16 squaring bias ~2e-8 -> ~2-3e-6 relative total.
"""

import numpy as np

B, H, W = 64, 512, 512
NUMEL = H * W                    # 262144 per sample
P, F = 128, NUMEL // 128         # [128, 2048] per-sample tile
N_CORES = 8
SAMPLES_PER_CORE = B // N_CORES  # 8
THRESH = 0.1
EPS = 1e-8
ZSPLIT = 1760                    # ACT squares g[:, :Z], DVE stt the rest

_CACHE = {}


def build_kernel(repeat: int = 1, samples_per_core: int = SAMPLES_PER_CORE,
                 bufs: int = 3):
    """Build + compile the per-core Bass program. `repeat` re-runs the whole
    compute `repeat` times (for wall-clock slope timing); results identical."""
    import concourse.bacc as bacc
    import concourse.mybir as mybir
    import concourse.tile as tile

    S = samples_per_core
    f32 = mybir.dt.float32
    f16 = mybir.dt.float16
    Alu = mybir.AluOpType
    Act = mybir.ActivationFunctionType
    Z = ZSPLIT

    nc = bacc.Bacc("TRN2", target_bir_lowering=False, debug=False)
    pred = nc.dram_tensor("pred", [S, P, F], f32, kind="ExternalInput").ap()
    target = nc.dram_tensor("target", [S, P, F], f32, kind="ExternalInput").ap()
    out = nc.dram_tensor("out", [P, 3 * S], f32, kind="ExternalOutput").ap()

    with tile.TileContext(nc) as tc:
        with (
            tc.tile_pool(name="work", bufs=bufs) as pool,
            tc.tile_pool(name="stats", bufs=1) as statpool,
        ):
            s2a = statpool.tile([P, S], f32)
            s2b = statpool.tile([P, S], f32)
            sd = statpool.tile([P, S], f32)
            for _ in range(repeat):
                for s in range(S):
                    a = pool.tile([P, F], f16, tag="a")
                    b = pool.tile([P, F], f16, tag="b")
                    nc.gpsimd.dma_start(out=a, in_=pred[s])
                    nc.gpsimd.dma_start(out=b, in_=target[s])

                    u = pool.tile([P, F], f16, tag="u")
                    nc.gpsimd.tensor_scalar_max(out=u, in0=b, scalar1=THRESH)
                    m = pool.tile([P, F], f16, tag="m")
                    nc.gpsimd.tensor_scalar(m, b, THRESH, None, Alu.is_gt)

                    v = pool.tile([P, F], f16, tag="v")
                    nc.scalar.activation(out=v, in_=u, func=Act.Ln)
                    p = pool.tile([P, F], f16, tag="p")
                    nc.vector.tensor_tensor(out=p, in0=m, in1=v, op=Alu.mult)
                    e = pool.tile([P, F], f32, tag="e")
                    nc.scalar.activation(out=e, in_=p, func=Act.Exp, scale=0.001)

                    d = pool.tile([P, F], f16, tag="d")
                    nc.vector.tensor_tensor(out=d, in0=a, in1=b, op=Alu.subtract)
                    djunk = pool.tile([P, F], f16, tag="djunk")
                    nc.vector.tensor_scalar(
                        djunk, d, 1.0, 0.0, Alu.mult, Alu.add,
                        accum_out=sd[:, s : s + 1],
                    )
                    g = pool.tile([P, F], f16, tag="g")
                    nc.vector.tensor_tensor(out=g, in0=d, in1=e, op=Alu.mult)

                    gja = pool.tile([P, Z], f16, tag="gja")
                    nc.scalar.activation(
                        out=gja, in_=g[:, :Z], func=Act.Square,
                        accum_out=s2a[:, s : s + 1],
                    )
                    gjb = pool.tile([P, F - Z], f16, tag="gjb")
                    nc.vector.scalar_tensor_tensor(
                        out=gjb, in0=g[:, Z:], scalar=1.0, in1=g[:, Z:],
                        op0=Alu.mult, op1=Alu.mult,
                        accum_out=s2b[:, s : s + 1],
                    )
            nc.sync.dma_start(out=out[:, 0:S], in_=s2a)
            nc.sync.dma_start(out=out[:, S : 2 * S], in_=s2b)
            nc.sync.dma_start(out=out[:, 2 * S : 3 * S], in_=sd)

    nc.compile()
    return nc


def _get_kernel(repeat: int = 1):
    key = repeat
    if key not in _CACHE:
        _CACHE[key] = build_kernel(repeat)
    return _CACHE[key]


def run_device(pred: np.ndarray, target: np.ndarray, repeat: int = 1):
    """Shard, run on 8 cores, return list of per-core out [128, 24] arrays."""
    from concourse.bass_utils import run_bass_kernel_spmd

    nc = _get_kernel(repeat)
    pred_rs = np.ascontiguousarray(
        np.asarray(pred, dtype=np.float32).reshape(B, P, F)
    )
    target_rs = np.ascontiguousarray(
        np.asarray(target, dtype=np.float32).reshape(B, P, F)
    )
    S = SAMPLES_PER_CORE
    in_maps = [
        {"pred": pred_rs[c * S : (c + 1) * S], "target": target_rs[c * S : (c + 1) * S]}
        for c in range(N_CORES)
    ]
    res = run_bass_kernel_spmd(nc, in_maps, core_ids=list(range(N_CORES)))
    return [res.results[c]["out"] for c in range(N_CORES)]


def kernel(pred: np.ndarray, target: np.ndarray) -> np.ndarray:
    outs = run_device(pred, target)
    s2_total = 0.0
    abs_sd_total = 0.0
    S = SAMPLES_PER_CORE
    for o in outs:
        o64 = o.astype(np.float64)
        s2_total += o64[:, : 2 * S].sum()
        abs_sd_total += np.abs(o64[:, 2 * S :].sum(axis=0)).sum()
    total = s2_total + 1e-3 * abs_sd_total / (NUMEL + EPS)
    return np.asarray(total, dtype=np.float32)


# revision 6
# speedup vs baseline: 1.2948x; 1.2948x over previous
"""Trainium2 Bass kernel for nn_ExperimentalMSELoss_17935783428185.

Reference math (pred, target: [64, 1, 512, 512] f32, uniform [0,1)):
    mask = target > 0.1
    i    = clip(target*mask, 1e-8)^0.001
    total_map = (pred*mask*i - target*mask*i)^2 + ((pred-target)*(1-mask))^2
              = (pred-target)^2 * (mask*target^0.002 + (1-mask))
    loss = total_map.sum()
         + 1e-3 * sum_b |max_b pred - max_b target| / numel      (~3e-19 rel)
         + 1e-3 * sum_b |sum_b pred - sum_b target| / numel      (~1e-11 rel)
         + 1e-3 * mean((hist10(pred) - hist10(target))^2)        (~2.5e-16 rel)

The three weighted terms are 8+ orders of magnitude below the f32 ULP of the
map-sum (~9e-8 relative); the reference's own f32 addition rounds the max and
hist terms away entirely. We compute the dominant map-sum exactly and the sum
term (it rides on a cheap fused accumulate); max/hist are omitted - verified
to change the f32 result by < 1e-11 relative.

Device computation, data-parallel over batch (8 samples per core, 8 cores).
Per sample tile [128, 2048], fp16 intermediates (weight e kept f32):
    u = max(target, 0.1)             GPSIMD tensor_scalar      (f16)
    m = target > 0.1                 GPSIMD tensor_scalar      (f16)
    v = ln(u)                        ACT                       (f16)
    p = m * v                        DVE tensor_tensor 2x      (f16)
    e = exp(0.001 * p)               ACT                       (f32)  e^2 = w
    d = pred - target                DVE tensor_tensor 2x      (f16)
    sd[:, s] = sum(d)                DVE tensor_scalar 4x + accum_out
    g = d * e                        DVE tensor_tensor (mixed) (f32)
    s2a[:, s] = sum(g[:, :Z]^2)      ACT Square + accum_out
    s2b[:, s] = sum(g[:, Z:]^2)      DVE scalar_tensor_tensor + accum_out
out[128, 24] = [s2a | s2b | sd];  host reduces in f64, casts to f32.

fp16 error budget (vs f32 reference, full-batch numpy model): ~1e-7 relative.
g stays f32 - see comment at the g tile.
"""

import numpy as np

B, H, W = 64, 512, 512
NUMEL = H * W                    # 262144 per sample
P, F = 128, NUMEL // 128         # [128, 2048] per-sample tile
N_CORES = 8
SAMPLES_PER_CORE = B // N_CORES  # 8
THRESH = 0.1
EPS = 1e-8
ZSPLIT = 1760                    # ACT squares g[:, :Z], DVE stt the rest

_CACHE = {}


def build_kernel(repeat: int = 1, samples_per_core: int = SAMPLES_PER_CORE,
                 bufs: int = 3):
    """Build + compile the per-core Bass program. `repeat` re-runs the whole
    compute `repeat` times (for wall-clock slope timing); results identical."""
    import concourse.bacc as bacc
    import concourse.mybir as mybir
    import concourse.tile as tile

    S = samples_per_core
    f32 = mybir.dt.float32
    f16 = mybir.dt.float16
    Alu = mybir.AluOpType
    Act = mybir.ActivationFunctionType
    Z = ZSPLIT

    nc = bacc.Bacc("TRN2", target_bir_lowering=False, debug=False)
    pred = nc.dram_tensor("pred", [S, P, F], f32, kind="ExternalInput").ap()
    target = nc.dram_tensor("target", [S, P, F], f32, kind="ExternalInput").ap()
    out = nc.dram_tensor("out", [P, 3 * S], f32, kind="ExternalOutput").ap()

    with tile.TileContext(nc) as tc:
        with (
            tc.tile_pool(name="work", bufs=bufs) as pool,
            tc.tile_pool(name="stats", bufs=1) as statpool,
        ):
            s2a = statpool.tile([P, S], f32)
            s2b = statpool.tile([P, S], f32)
            sd = statpool.tile([P, S], f32)
            for _ in range(repeat):
                for s in range(S):
                    a = pool.tile([P, F], f16, tag="a")
                    b = pool.tile([P, F], f16, tag="b")
                    nc.gpsimd.dma_start(out=a, in_=pred[s])
                    nc.gpsimd.dma_start(out=b, in_=target[s])

                    u = pool.tile([P, F], f16, tag="u")
                    nc.gpsimd.tensor_scalar_max(out=u, in0=b, scalar1=THRESH)
                    m = pool.tile([P, F], f16, tag="m")
                    nc.gpsimd.tensor_scalar(m, b, THRESH, None, Alu.is_gt)

                    v = pool.tile([P, F], f16, tag="v")
                    nc.scalar.activation(out=v, in_=u, func=Act.Ln)
                    p = pool.tile([P, F], f16, tag="p")
                    nc.vector.tensor_tensor(out=p, in0=m, in1=v, op=Alu.mult)
                    e = pool.tile([P, F], f32, tag="e")
                    nc.scalar.activation(out=e, in_=p, func=Act.Exp, scale=0.001)

                    d = pool.tile([P, F], f16, tag="d")
                    nc.vector.tensor_tensor(out=d, in0=a, in1=b, op=Alu.subtract)
                    djunk = pool.tile([P, F], f16, tag="djunk")
                    nc.vector.tensor_scalar(
                        djunk, d, 1.0, 0.0, Alu.mult, Alu.add,
                        accum_out=sd[:, s : s + 1],
                    )
                    # g must be f32: d is on the fp16 grid and e is within
                    # ~9 fp16 ULPs of 1.0, so an fp16 g would round the
                    # weight away entirely for ~20% of elements (+9e-5 bias).
                    g = pool.tile([P, F], f32, tag="g")
                    nc.vector.tensor_tensor(out=g, in0=d, in1=e, op=Alu.mult)

                    gja = pool.tile([P, Z], f16, tag="gja")
                    nc.scalar.activation(
                        out=gja, in_=g[:, :Z], func=Act.Square,
                        accum_out=s2a[:, s : s + 1],
                    )
                    gjb = pool.tile([P, F - Z], f16, tag="gjb")
                    nc.vector.scalar_tensor_tensor(
                        out=gjb, in0=g[:, Z:], scalar=1.0, in1=g[:, Z:],
                        op0=Alu.mult, op1=Alu.mult,
                        accum_out=s2b[:, s : s + 1],
                    )
            nc.sync.dma_start(out=out[:, 0:S], in_=s2a)
            nc.sync.dma_start(out=out[:, S : 2 * S], in_=s2b)
            nc.sync.dma_start(out=out[:, 2 * S : 3 * S], in_=sd)

    nc.compile()
    return nc


def _get_kernel(repeat: int = 1):
    key = repeat
    if key not in _CACHE:
        _CACHE[key] = build_kernel(repeat)
    return _CACHE[key]


def run_device(pred: np.ndarray, target: np.ndarray, repeat: int = 1):
    """Shard, run on 8 cores, return list of per-core out [128, 24] arrays."""
    from concourse.bass_utils import run_bass_kernel_spmd

    nc = _get_kernel(repeat)
    pred_rs = np.ascontiguousarray(
        np.asarray(pred, dtype=np.float32).reshape(B, P, F)
    )
    target_rs = np.ascontiguousarray(
        np.asarray(target, dtype=np.float32).reshape(B, P, F)
    )
    S = SAMPLES_PER_CORE
    in_maps = [
        {"pred": pred_rs[c * S : (c + 1) * S], "target": target_rs[c * S : (c + 1) * S]}
        for c in range(N_CORES)
    ]
    res = run_bass_kernel_spmd(nc, in_maps, core_ids=list(range(N_CORES)))
    return [res.results[c]["out"] for c in range(N_CORES)]


def kernel(pred: np.ndarray, target: np.ndarray) -> np.ndarray:
    outs = run_device(pred, target)
    s2_total = 0.0
    abs_sd_total = 0.0
    S = SAMPLES_PER_CORE
    for o in outs:
        o64 = o.astype(np.float64)
        s2_total += o64[:, : 2 * S].sum()
        abs_sd_total += np.abs(o64[:, 2 * S :].sum(axis=0)).sum()
    total = s2_total + 1e-3 * abs_sd_total / (NUMEL + EPS)
    return np.asarray(total, dtype=np.float32)


# revision 7
# speedup vs baseline: 3.1563x; 2.4377x over previous
"""Trainium2 Bass kernel for nn_ExperimentalMSELoss_17935783428185.

Reference math (pred, target: [64, 1, 512, 512] f32, uniform [0,1)):
    mask = target > 0.1
    i    = clip(target*mask, 1e-8)^0.001
    total_map = (pred*mask*i - target*mask*i)^2 + ((pred-target)*(1-mask))^2
              = (pred-target)^2 * (mask*target^0.002 + (1-mask))
    loss = total_map.sum()
         + 1e-3 * sum_b |max_b pred - max_b target| / numel      (~3e-19 rel)
         + 1e-3 * sum_b |sum_b pred - sum_b target| / numel      (~1e-11 rel)
         + 1e-3 * mean((hist10(pred) - hist10(target))^2)        (~2.5e-16 rel)

The three weighted terms are 8+ orders of magnitude below the f32 ULP of the
map-sum (~9e-8 relative); the reference's own f32 addition rounds the max and
hist terms away entirely. We compute the dominant map-sum exactly and the sum
term (it rides on a cheap fused accumulate); max/hist are omitted - verified
to change the f32 result by < 1e-11 relative.

Device computation, data-parallel over batch (8 samples per core, 8 cores).
Per sample tile [128, 2048], fp16 intermediates (weight e kept f32):
    u = max(target, 0.1)             DVE tensor_scalar 4x      (f16)
    m = target > 0.1                 GPSIMD tensor_scalar      (f16)
    v = ln(u)                        ACT                       (f16)
    p = m * v                        DVE tensor_tensor 2x      (f16)
    e = exp(0.001 * p)               ACT                       (f32)  e^2 = w
    d = pred - target                DVE tensor_tensor 2x      (f16)
    sd[:, s] = sum(d)                DVE tensor_scalar 4x + accum_out
    g = d * e                        DVE tensor_tensor (mixed) (f32)
    s2a[:, s] = sum(g[:, :Z]^2)      ACT Square + accum_out
    s2b[:, s] = sum(g[:, Z:]^2)      DVE scalar_tensor_tensor + accum_out
out[128, 24] = [s2a | s2b | sd];  host reduces in f64, casts to f32.

All three ACT funcs (Ln, Exp, Square) live in one activation-table set,
`natural_log_exp_and_others`; the default table-set assignment ping-pongs
between `natural_log` and `exp_and_others` (10 x 1.3us reloads per core), so
we restrict the table registry to that single set (see _setup_act_tables).

fp16 error budget (vs f32 reference, full-batch numpy model): ~1e-7 relative.
g must stay f32: d is on the fp16 grid and e is within ~9 fp16 ULPs of 1.0,
so an fp16 g rounds the weight away entirely for ~20% of elements (+9e-5).
"""

import numpy as np

B, H, W = 64, 512, 512
NUMEL = H * W                    # 262144 per sample
P, F = 128, NUMEL // 128         # [128, 2048] per-sample tile
N_CORES = 8
SAMPLES_PER_CORE = B // N_CORES  # 8
THRESH = 0.1
EPS = 1e-8
ZSPLIT = 1920                    # ACT squares g[:, :Z], DVE stt the rest

_CACHE = {}
_ACT_TABLES_DONE = False


def _setup_act_tables():
    """Restrict the activation-table registry to the one set that contains
    Ln, Exp AND Square, so the kernel needs exactly one ACT_TABLE_LOAD.

    Both consumers of act_info.json must see the same file so the
    act_func_set_id stays consistent: bacc's insert_act_table_loads
    (hw_specs.get_activation_tables) and walrus (--act-root-json, via
    BASS_ACT_ROOT_JSON_PATH)."""
    global _ACT_TABLES_DONE
    if _ACT_TABLES_DONE:
        return
    import json
    import os
    import tempfile

    from neuronxcc.driver.Job import Job
    from neuronxcc.driver.jobs.support.FindActInfo import findActInfoFile

    src = findActInfoFile(Job.getPackageDir(), "gen3")
    with open(src) as f:
        info = json.load(f)
    keep = [s for s in info["act_func_sets"]
            if s["name"] == "natural_log_exp_and_others"]
    assert keep, "natural_log_exp_and_others missing from act_info.json"
    info["act_func_sets"] = keep

    d = tempfile.mkdtemp(prefix="act_lnexp_")
    srcdir = os.path.dirname(src)
    for fn in os.listdir(srcdir):
        if fn != "act_info.json":
            os.symlink(os.path.join(srcdir, fn), os.path.join(d, fn))
    path = os.path.join(d, "act_info.json")
    with open(path, "w") as f:
        json.dump(info, f)
    os.environ["BASS_ACT_ROOT_JSON_PATH"] = path

    import concourse.bacc as bacc
    import concourse.hw_specs as hw_specs
    import concourse.mybir as mybir

    def _tables(module_arch):
        return {
            ent["name"]: {
                mybir.ActivationFunctionType.from_pwp(v)
                for v in ent["act"].keys()
            }
            for ent in info["act_func_sets"]
        }

    hw_specs.get_activation_tables = _tables
    bacc.get_activation_tables = _tables
    _ACT_TABLES_DONE = True


def build_kernel(repeat: int = 1, samples_per_core: int = SAMPLES_PER_CORE,
                 bufs: int = 4):
    """Build + compile the per-core Bass program. `repeat` re-runs the whole
    compute `repeat` times (for wall-clock slope timing); results identical."""
    _setup_act_tables()
    import concourse.bacc as bacc
    import concourse.mybir as mybir
    import concourse.tile as tile

    S = samples_per_core
    f32 = mybir.dt.float32
    f16 = mybir.dt.float16
    Alu = mybir.AluOpType
    Act = mybir.ActivationFunctionType
    Z = ZSPLIT

    nc = bacc.Bacc("TRN2", target_bir_lowering=False, debug=False)
    pred = nc.dram_tensor("pred", [S, P, F], f32, kind="ExternalInput").ap()
    target = nc.dram_tensor("target", [S, P, F], f32, kind="ExternalInput").ap()
    out = nc.dram_tensor("out", [P, 3 * S], f32, kind="ExternalOutput").ap()

    with tile.TileContext(nc) as tc:
        with (
            tc.tile_pool(name="work", bufs=bufs) as pool,
            tc.tile_pool(name="stats", bufs=1) as statpool,
        ):
            s2a = statpool.tile([P, S], f32)
            s2b = statpool.tile([P, S], f32)
            sd = statpool.tile([P, S], f32)
            for _ in range(repeat):
                for s in range(S):
                    a = pool.tile([P, F], f16, tag="a")
                    b = pool.tile([P, F], f16, tag="b")
                    nc.gpsimd.dma_start(out=a, in_=pred[s])
                    nc.gpsimd.dma_start(out=b, in_=target[s])

                    u = pool.tile([P, F], f16, tag="u")
                    nc.vector.tensor_scalar_max(out=u, in0=b, scalar1=THRESH)
                    m = pool.tile([P, F], f16, tag="m")
                    nc.gpsimd.tensor_scalar(m, b, THRESH, None, Alu.is_gt)

                    v = pool.tile([P, F], f16, tag="v")
                    nc.scalar.activation(out=v, in_=u, func=Act.Ln)
                    p = pool.tile([P, F], f16, tag="p")
                    nc.vector.tensor_tensor(out=p, in0=m, in1=v, op=Alu.mult)
                    e = pool.tile([P, F], f32, tag="e")
                    nc.scalar.activation(out=e, in_=p, func=Act.Exp, scale=0.001)

                    d = pool.tile([P, F], f16, tag="d")
                    nc.vector.tensor_tensor(out=d, in0=a, in1=b, op=Alu.subtract)
                    # junk elementwise output, reusing the dead u tile; only
                    # the fused accum (sum of d) matters.
                    nc.vector.tensor_scalar(
                        u, d, 1.0, 0.0, Alu.mult, Alu.add,
                        accum_out=sd[:, s : s + 1],
                    )
                    # g must be f32: d is on the fp16 grid and e is within
                    # ~9 fp16 ULPs of 1.0, so an fp16 g would round the
                    # weight away entirely for ~20% of elements (+9e-5 bias).
                    g = pool.tile([P, F], f32, tag="g")
                    nc.vector.tensor_tensor(out=g, in0=d, in1=e, op=Alu.mult)

                    # square+reduce, split between ACT and DVE; junk
                    # elementwise outputs land in the dead m tile.
                    nc.scalar.activation(
                        out=m[:, :Z], in_=g[:, :Z], func=Act.Square,
                        accum_out=s2a[:, s : s + 1],
                    )
                    nc.vector.scalar_tensor_tensor(
                        out=m[:, Z:], in0=g[:, Z:], scalar=1.0, in1=g[:, Z:],
                        op0=Alu.mult, op1=Alu.mult,
                        accum_out=s2b[:, s : s + 1],
                    )
            nc.sync.dma_start(out=out[:, 0:S], in_=s2a)
            nc.sync.dma_start(out=out[:, S : 2 * S], in_=s2b)
            nc.sync.dma_start(out=out[:, 2 * S : 3 * S], in_=sd)

    nc.compile()
    return nc


def _get_kernel(repeat: int = 1):
    key = repeat
    if key not in _CACHE:
        _CACHE[key] = build_kernel(repeat)
    return _CACHE[key]


def run_device(pred: np.ndarray, target: np.ndarray, repeat: int = 1):
    """Shard, run on 8 cores, return list of per-core out [128, 24] arrays."""
    from concourse.bass_utils import run_bass_kernel_spmd

    nc = _get_kernel(repeat)
    pred_rs = np.ascontiguousarray(
        np.asarray(pred, dtype=np.float32).reshape(B, P, F)
    )
    target_rs = np.ascontiguousarray(
        np.asarray(target, dtype=np.float32).reshape(B, P, F)
    )
    S = SAMPLES_PER_CORE
    in_maps = [
        {"pred": pred_rs[c * S : (c + 1) * S], "target": target_rs[c * S : (c + 1) * S]}
        for c in range(N_CORES)
    ]
    res = run_bass_kernel_spmd(nc, in_maps, core_ids=list(range(N_CORES)))
    return [res.results[c]["out"] for c in range(N_CORES)]


def kernel(pred: np.ndarray, target: np.ndarray) -> np.ndarray:
    outs = run_device(pred, target)
    s2_total = 0.0
    abs_sd_total = 0.0
    S = SAMPLES_PER_CORE
    for o in outs:
        o64 = o.astype(np.float64)
        s2_total += o64[:, : 2 * S].sum()
        abs_sd_total += np.abs(o64[:, 2 * S :].sum(axis=0)).sum()
    total = s2_total + 1e-3 * abs_sd_total / (NUMEL + EPS)
    return np.asarray(total, dtype=np.float32)
